# revision 47
# baseline (speedup 1.0000x reference)
"""CoevolExtractor fused kernel for 8x trn2 NeuronCores (Bass/Tile).

Computation (reference):
    pair[b,i,l,j,m] = sum_n x_down[b,n,i,j] * x_down_w[b,n,l,m]
    pair = LayerNorm_{(j,m)}(pair) * a_2 + b_2        (eps=1e-5, biased var)
    out  = pair @ W + b                               # (1, L, L, 128)

Strategy: shard i (first residue axis) across 8 cores (24 i's each).
Per core:
  pair_slab = A_slab^T @ B  (fp32r matmuls, K=256) in layout [(i4,j) x (l,m)]
  LayerNorm folded into the projection:
     out[t,f] = invstd[t] * ( (pair @ W')[t,f] + s[f]*(-mean[t]) + bconst[f]*std[t] )
  where W' = a_2*W, s = sum_c W', bconst = b_2@W + b  (host-prepped weights)
  mean via factorized input row-sums (sA^T sB); sumsq via DVE squares +
  in-place bf16 fold-tree over m + per-row-tile indicator matmuls over j.
  Linear via m-strided matmuls: K=32 j-contraction, 4 concurrent row-groups
  (tile_position), rank-1 corrections as K=32 matmuls at matching positions.
  Output scaled by broadcast invstd (gpsimd+DVE), written [f, t]; the host
  unshard reorders to (B, L, L, F).
"""

import os
from contextlib import ExitStack

import ml_dtypes
import numpy as np

import concourse.bass as bass
import concourse.tile as tile
from concourse import bacc, mybir
from concourse.bass_utils import run_bass_kernel_spmd

F32 = mybir.dt.float32
F32R = mybir.dt.float32r
BF16 = mybir.dt.bfloat16

B, N, L, J = 1, 256, 192, 32
D2 = J * J          # 1024
F = 128             # n_feat_out
NCORES = 8
LI = L // NCORES    # 24 i's per core
NK = N // 128       # 2 contraction k-tiles
NRT = LI * J // 128  # 6 row tiles of (i4, j)
CB = 512            # pair col-block width
NCB = L * J // CB   # 12 col blocks
NRP = NRT // 2      # 3 row-tile pairs
EPS = 1e-5
AX = mybir.AxisListType
ALU = mybir.AluOpType
ACTF = mybir.ActivationFunctionType


def build_kernel(ctx: ExitStack, tc: tile.TileContext, xa, xb, wrep, sb2, bones, y):
    nc = tc.nc

    const = ctx.enter_context(tc.tile_pool(name="const", bufs=1))
    bpool = ctx.enter_context(tc.tile_pool(name="b2", bufs=1))
    prpool = ctx.enter_context(tc.tile_pool(name="pairsb", bufs=1))
    work = ctx.enter_context(tc.tile_pool(name="work", bufs=1))
    bank = ctx.enter_context(tc.tile_pool(name="bank", bufs=6, space="PSUM"))
    statp = ctx.enter_context(tc.tile_pool(name="statp", bufs=1, space="PSUM"))
    pslp = bank  # share the 6 rotating bank slots between pair tiles and psl

    # ---- constants / inputs to SBUF ----
    a_t = []
    for k in range(NK):
        at = const.tile([128, LI * J], F32R, tag=f"a{k}")
        nc.sync.dma_start(at[:], xa[k * 128:(k + 1) * 128, :])
        a_t.append(at)
    wrep_t = const.tile([128, J * F], BF16, tag="wrep")
    nc.sync.dma_start(wrep_t[:], wrep[:])
    sb2_t = const.tile([128, F], BF16, tag="sb2")
    nc.sync.dma_start(sb2_t[:], sb2[:])
    bones_t = const.tile([128, NRT * LI], F32, tag="bones")
    nc.sync.dma_start(bones_t[:], bones[:])

    b_t = [[None] * NCB for _ in range(NK)]
    for cb in range(NCB):
        for k in range(NK):
            bt = bpool.tile([128, CB], F32R, tag=f"b{k}_{cb}")
            nc.sync.dma_start(bt[:], xb[k * 128:(k + 1) * 128, cb * CB:(cb + 1) * CB])
            b_t[k][cb] = bt

    pair_sb = [prpool.tile([128, 2 * L * J], BF16, tag=f"prp{rp}", name=f"prp{rp}")
               for rp in range(NRP)]

    eps24 = work.tile([LI, 1], F32, tag="eps24")
    nc.gpsimd.memset(eps24[:], EPS)
    # staged per-t rows, replicated at partitions {32g, 32g+1}, zero-padded K=32
    stage2 = work.tile([128, LI * L], BF16, tag="stage2")
    nc.gpsimd.memset(stage2[:], 0.0)
    stage_inv = work.tile([1, LI * L], F32, tag="stage_inv")

    # factorized mean: sA[n,i] = sum_j A, sB[n,l] = sum_m B; mean = sA^T sB / D2
    sa_t = []
    for k in range(NK):
        sa = work.tile([128, LI], F32, tag=f"sa{k}")
        nc.vector.tensor_reduce(
            sa[:], a_t[k][:].rearrange("p (i j) -> p i j", j=J),
            axis=AX.X, op=ALU.add)
        sa_t.append(sa)
    sb_t = []
    for k in range(NK):
        sb = work.tile([128, L], F32, tag=f"sb{k}")
        for cb in range(NCB):
            nc.vector.tensor_reduce(
                sb[:, cb * 16:(cb + 1) * 16],
                b_t[k][cb][:].rearrange("p (l m) -> p l m", m=J),
                axis=AX.X, op=ALU.add)
        sb_t.append(sb)
    mean_ps = statp.tile([LI, L], F32, tag="mean_ps")
    for k in range(NK):
        nc.tensor.matmul(mean_ps[:], sa_t[k][:], sb_t[k][:],
                         start=(k == 0), stop=(k == NK - 1))

    # ---- phase C m-loop emitter (rank-1 + epilogue emitted separately) ----
    no_rank1 = bool(os.environ.get("COEVOL_NO_RANK1"))
    rank1_only = bool(os.environ.get("COEVOL_RANK1_ONLY"))
    psl_all = {}

    def emit_m_loop(rp):
        psl = [bank.tile([128, CB], F32, tag="bank", name=f"psl{rp}_{g}")[:, 0:2 * L]
               for g in range(4)]
        psl_all[rp] = psl
        prp4 = pair_sb[rp][:].rearrange("p (r l m) -> p r l m", r=2, m=J)
        for m in ([] if rank1_only else range(J)):
            for g in range(4):
                nc.tensor.matmul(
                    psl[g][:],
                    wrep_t[32 * g:32 * (g + 1), m * F:(m + 1) * F],
                    prp4[32 * g:32 * (g + 1), :, :, m],
                    start=(m == 0), stop=(no_rank1 and m == J - 1),
                    tile_position=(32 * g, 0),
                    skip_group_check=True)

    # ---- phase A: pair matmuls + bf16 copies + sumsq reduction ----
    ssq_ps = statp.tile([LI, L], F32, tag="ssq_ps")
    for rt in range(NRT):
        rp, rt2 = rt // 2, rt % 2
        for cb in range(NCB):
            pp = bank.tile([128, CB], F32, tag="bank")
            for k in range(NK):
                nc.tensor.matmul(
                    pp[:],
                    a_t[k][:, rt * 128:(rt + 1) * 128],
                    b_t[k][cb][:],
                    start=(k == 0),
                    stop=(k == NK - 1),
                )
            # psum -> sbuf bf16 copy (Linear input)
            nc.scalar.activation(
                pair_sb[rp][:, rt2 * L * J + cb * CB: rt2 * L * J + (cb + 1) * CB],
                pp[:], ACTF.Copy)
        # squares (bf16) on DVE; in-place fold-tree m-reduce (bf16) on DVE
        sq_bf = work.tile([128, L * J], BF16, tag="sq_bf", bufs=2)
        pslice = pair_sb[rp][:, rt2 * L * J:(rt2 + 1) * L * J]
        nc.vector.scalar_tensor_tensor(
            sq_bf[:], pslice, 1.0, pslice, op0=ALU.mult, op1=ALU.mult)
        sqv = sq_bf[:].rearrange("p (l m) -> p l m", m=J)
        half = J // 2
        while half > 1:
            nc.vector.tensor_add(
                sqv[:, :, 0:half], sqv[:, :, 0:half], sqv[:, :, half:2 * half])
            half //= 2
        ssq_part = work.tile([128, L], F32, tag="ssq_part", bufs=2)
        nc.vector.tensor_add(ssq_part[:], sqv[:, :, 0], sqv[:, :, 1])
        nc.tensor.matmul(ssq_ps[:], bones_t[:, rt * LI:(rt + 1) * LI], ssq_part[:],
                         start=(rt == 0), stop=(rt == NRT - 1),
                         skip_group_check=True)
        if rt == 1 and os.environ.get("COEVOL_ILV"):
            # overlap rp0's Linear m-loop with the remaining pair work
            emit_m_loop(0)

    # ---- finalize stats wholesale on [24, L] ----
    mean24 = work.tile([LI, L], F32, tag="mean24")
    nc.vector.tensor_scalar_mul(mean24[:], mean_ps[:], 1.0 / D2)
    mean2 = work.tile([LI, L], F32, tag="mean2")
    nc.vector.tensor_mul(mean2[:], mean24[:], mean24[:])
    var24 = work.tile([LI, L], F32, tag="var24")
    nc.vector.scalar_tensor_tensor(
        var24[:], ssq_ps[:], 1.0 / D2, mean2[:], op0=ALU.mult, op1=ALU.subtract)
    std24 = work.tile([LI, L], F32, tag="std24")
    nc.scalar.activation(std24[:], var24[:], ACTF.Sqrt, bias=eps24[:])
    invstd24 = work.tile([LI, L], F32, tag="invstd24")
    nc.vector.reciprocal(invstd24[:], std24[:])
    mneg24 = work.tile([LI, L], BF16, tag="mneg24")
    nc.vector.tensor_scalar_mul(mneg24[:], mean24[:], -1.0)
    stdbf24 = work.tile([LI, L], BF16, tag="stdbf24")
    nc.vector.tensor_copy(stdbf24[:], std24[:])
    for g in range(4):
        nc.sync.dma_start(
            stage2[32 * g:32 * g + 1, 0:LI * L].rearrange("o (i l) -> o i l", i=LI),
            mneg24[:])
        nc.sync.dma_start(
            stage2[32 * g + 1:32 * g + 2, 0:LI * L].rearrange("o (i l) -> o i l", i=LI),
            stdbf24[:])
    nc.sync.dma_start(stage_inv[0:1, :].rearrange("o (i l) -> o i l", i=LI), invstd24[:])

    if os.environ.get("COEVOL_PHASE") == "A":
        # keep phase A live through its outputs; fill the rest with zeros
        nc.sync.dma_start(y[0:1, :], stage_inv[:])
        nc.sync.dma_start(y[1:3, 0:LI * L // 2], stage2[0:2, :].bitcast(F32)[:, 0:LI * L // 2])
        nc.sync.dma_start(y[3:128, :],
                          pair_sb[0][:].bitcast(F32)[3:128, 0:LI * L])
        return

    # ---- phase C: remaining m-loops + rank-1 + invstd scale ----
    st4 = stage2[:].rearrange("p (h g l) -> p h g l", g=4, l=L)
    for rp in range(NRP):
        if rp not in psl_all:
            emit_m_loop(rp)
        psl = psl_all[rp]
        # rank-1 corrections: s x (-mean) + bconst x std
        if not no_rank1:
            for g in range(4):
                nc.tensor.matmul(
                    psl[g][:],
                    sb2_t[32 * g:32 * (g + 1), :],
                    st4[32 * g:32 * (g + 1), 2 * rp:2 * rp + 2, g, :],
                    start=rank1_only, stop=True, tile_position=(32 * g, 0),
                    skip_group_check=True)
        # epilogue: scale columns by invstd[t], write out in [f, t] layout
        for g in range(4):
            out_sb = work.tile([128, 2 * L], F32, tag="out_sb", bufs=3)
            if os.environ.get("COEVOL_NO_PBCAST"):
                nc.vector.tensor_copy(out_sb[:], psl[g][:])
            else:
                inv_bc = work.tile([128, 2 * L], F32, tag="inv_bc", bufs=3)
                for rt2 in range(2):
                    i = (2 * rp + rt2) * 4 + g
                    nc.gpsimd.partition_broadcast(
                        inv_bc[:, rt2 * L:(rt2 + 1) * L],
                        stage_inv[0:1, i * L:(i + 1) * L])
                nc.vector.tensor_mul(out_sb[:], psl[g][:], inv_bc[:])
            for rt2 in range(2):
                i = (2 * rp + rt2) * 4 + g
                nc.sync.dma_start(y[:, i * L:(i + 1) * L], out_sb[:, rt2 * L:(rt2 + 1) * L])


def build_program():
    nc = bacc.Bacc("TRN2", target_bir_lowering=False, debug=False,
                   num_devices=NCORES)
    xa = nc.dram_tensor("xa", [N, LI * J], F32R, kind="ExternalInput").ap()
    xb = nc.dram_tensor("xb", [N, L * J], F32R, kind="ExternalInput").ap()
    wrep = nc.dram_tensor("wrep", [128, J * F], BF16, kind="ExternalInput").ap()
    sb2 = nc.dram_tensor("sb2", [128, F], BF16, kind="ExternalInput").ap()
    bones = nc.dram_tensor("bones", [128, NRT * LI], F32, kind="ExternalInput").ap()
    y = nc.dram_tensor("y", [F, LI * L], F32, kind="ExternalOutput").ap()

    reps = int(os.environ.get("COEVOL_REPS", "1"))
    with tile.TileContext(nc) as tc:
        for _ in range(reps):
            with ExitStack() as ctx:
                build_kernel(ctx, tc, xa, xb, wrep, sb2, bones, y)
    nc.compile()
    return nc


def host_inputs(x_down, x_down_w, a_2, b_2, W, b):
    """Host-side prep: reshapes + weight prepacking. Returns per-core input maps."""
    A2 = np.ascontiguousarray(x_down.reshape(N, L * J).astype(np.float32))
    B2 = np.ascontiguousarray(x_down_w.reshape(N, L * J).astype(np.float32))
    Wp = (a_2.astype(np.float64)[:, None] * W.astype(np.float64))
    s_row = Wp.sum(axis=0)
    bconst = b_2.astype(np.float64) @ W.astype(np.float64) + b.astype(np.float64)
    wrep = np.tile(Wp.reshape(J, J * F), (4, 1)).astype(ml_dtypes.bfloat16)
    sb2 = np.zeros((128, F), dtype=ml_dtypes.bfloat16)
    for g in range(4):
        sb2[32 * g] = s_row.astype(ml_dtypes.bfloat16)
        sb2[32 * g + 1] = bconst.astype(ml_dtypes.bfloat16)
    # per-row-tile j-reduction indicators: bones[:, rt*LI + i'] = 1 where the
    # partition belongs to group g and i' == 4*rt + g
    bones = np.zeros((128, NRT * LI), dtype=np.float32)
    for rt in range(NRT):
        for g in range(4):
            bones[32 * g:32 * (g + 1), rt * LI + 4 * rt + g] = 1.0
    in_maps = []
    for c in range(NCORES):
        in_maps.append({
            "xa": np.ascontiguousarray(A2[:, c * LI * J:(c + 1) * LI * J]),
            "xb": B2,
            "wrep": wrep,
            "sb2": sb2,
            "bones": bones,
        })
    return in_maps


_NC_CACHE = {}


def _get_program():
    if "nc" not in _NC_CACHE:
        _NC_CACHE["nc"] = build_program()
    return _NC_CACHE["nc"]


def kernel(**inputs) -> np.ndarray:
    nc = _get_program()
    inputs = {k: np.asarray(v) for k, v in inputs.items()}
    in_maps = host_inputs(**inputs)
    trace = bool(int(os.environ.get("COEVOL_TRACE", "0")))
    res = run_bass_kernel_spmd(nc, in_maps, list(range(NCORES)), trace=trace)
    if trace:
        _NC_CACHE["last_result"] = res
    # per-core y is [F, LI*L]; unshard to (B, L, L, F)
    slabs = [res.results[c]["y"].reshape(F, LI, L).transpose(1, 2, 0)
             for c in range(NCORES)]
    return np.concatenate(slabs, axis=0).reshape(B, L, L, F)


# revision 51
# speedup vs baseline: 1.0799x; 1.0799x over previous
"""CoevolExtractor fused kernel for 8x trn2 NeuronCores (Bass/Tile).

Computation (reference):
    pair[b,i,l,j,m] = sum_n x_down[b,n,i,j] * x_down_w[b,n,l,m]
    pair = LayerNorm_{(j,m)}(pair) * a_2 + b_2        (eps=1e-5, biased var)
    out  = pair @ W + b                               # (1, L, L, 128)

Strategy: shard i (first residue axis) across 8 cores (24 i's each).
Per core:
  pair_slab = A_slab^T @ B  (fp32r matmuls, K=256) in layout [(i4,j) x (l,m)]
  LayerNorm folded into the projection:
     out[t,f] = invstd[t] * ( (pair @ W')[t,f] + s[f]*(-mean[t]) + bconst[f]*std[t] )
  where W' = a_2*W, s = sum_c W', bconst = b_2@W + b  (host-prepped weights)
  mean via factorized input row-sums (sA^T sB); sumsq via DVE squares +
  in-place bf16 fold-tree over m + per-row-tile indicator matmuls over j.
  Linear via m-strided matmuls: K=32 j-contraction, 4 concurrent row-groups
  (tile_position), rank-1 corrections as K=32 matmuls at matching positions.
  Output scaled by broadcast invstd (gpsimd+DVE), written [f, t]; the host
  unshard reorders to (B, L, L, F).
"""

import os
from contextlib import ExitStack

import ml_dtypes
import numpy as np

import concourse.bass as bass
import concourse.tile as tile
from concourse import bacc, mybir
from concourse.bass_utils import run_bass_kernel_spmd

F32 = mybir.dt.float32
F32R = mybir.dt.float32r
BF16 = mybir.dt.bfloat16

B, N, L, J = 1, 256, 192, 32
D2 = J * J          # 1024
F = 128             # n_feat_out
NCORES = 8
LI = L // NCORES    # 24 i's per core
NK = N // 128       # 2 contraction k-tiles
NRT = LI * J // 128  # 6 row tiles of (i4, j)
CB = 512            # pair col-block width
NCB = L * J // CB   # 12 col blocks
NRP = NRT // 2      # 3 row-tile pairs
EPS = 1e-5
AX = mybir.AxisListType
ALU = mybir.AluOpType
ACTF = mybir.ActivationFunctionType


def build_kernel(ctx: ExitStack, tc: tile.TileContext, xa, xb, wrep, sb2, bones, y):
    nc = tc.nc

    const = ctx.enter_context(tc.tile_pool(name="const", bufs=1))
    bpool = ctx.enter_context(tc.tile_pool(name="b2", bufs=1))
    prpool = ctx.enter_context(tc.tile_pool(name="pairsb", bufs=1))
    work = ctx.enter_context(tc.tile_pool(name="work", bufs=1))
    bank = ctx.enter_context(tc.tile_pool(name="bank", bufs=7, space="PSUM"))
    statp = ctx.enter_context(tc.tile_pool(name="statp", bufs=1, space="PSUM"))
    pslp = bank  # share the 7 rotating bank slots between pair tiles and psl

    # ---- constants / inputs to SBUF ----
    a_t = []
    for k in range(NK):
        at = const.tile([128, LI * J], F32R, tag=f"a{k}")
        nc.sync.dma_start(at[:], xa[k * 128:(k + 1) * 128, :])
        a_t.append(at)
    b_t = [[None] * NCB for _ in range(NK)]
    for cb in range(NCB):
        for k in range(NK):
            bt = bpool.tile([128, CB], F32R, tag=f"b{k}_{cb}")
            nc.sync.dma_start(bt[:], xb[k * 128:(k + 1) * 128, cb * CB:(cb + 1) * CB])
            b_t[k][cb] = bt

    # weights/constants are needed late; emit after the activation loads
    wrep_t = const.tile([128, J * F], BF16, tag="wrep")
    nc.sync.dma_start(wrep_t[:], wrep[:])
    sb2_t = const.tile([128, F], BF16, tag="sb2")
    nc.sync.dma_start(sb2_t[:], sb2[:])
    bones_t = const.tile([128, NRT * LI], F32, tag="bones")
    nc.sync.dma_start(bones_t[:], bones[:])

    pair_sb = [prpool.tile([128, 2 * L * J], BF16, tag=f"prp{rp}", name=f"prp{rp}")
               for rp in range(NRP)]

    eps24 = work.tile([LI, 1], F32, tag="eps24")
    nc.gpsimd.memset(eps24[:], EPS)
    # staged per-t rows, replicated at partitions {32g, 32g+1}, zero-padded K=32
    stage2 = work.tile([128, LI * L], BF16, tag="stage2")
    nc.gpsimd.memset(stage2[:], 0.0)
    stage_inv = work.tile([1, LI * L], F32, tag="stage_inv")

    # factorized mean: sA[n,i] = sum_j A, sB[n,l] = sum_m B; mean = sA^T sB / D2
    sa_t = []
    for k in range(NK):
        sa = work.tile([128, LI], F32, tag=f"sa{k}")
        nc.vector.tensor_reduce(
            sa[:], a_t[k][:].rearrange("p (i j) -> p i j", j=J),
            axis=AX.X, op=ALU.add)
        sa_t.append(sa)
    sb_t = []
    for k in range(NK):
        sb = work.tile([128, L], F32, tag=f"sb{k}")
        for cb in range(NCB):
            nc.vector.tensor_reduce(
                sb[:, cb * 16:(cb + 1) * 16],
                b_t[k][cb][:].rearrange("p (l m) -> p l m", m=J),
                axis=AX.X, op=ALU.add)
        sb_t.append(sb)
    statb = statp.tile([LI, 2 * L], F32, tag="statb")
    mean_ps = statb[:, L:2 * L]
    ssq_ps = statb[:, 0:L]
    for k in range(NK):
        nc.tensor.matmul(mean_ps, sa_t[k][:], sb_t[k][:],
                         start=(k == 0), stop=(k == NK - 1),
                         skip_group_check=True)

    # ---- phase C m-loop emitter (rank-1 + epilogue emitted separately) ----
    no_rank1 = bool(os.environ.get("COEVOL_NO_RANK1"))
    rank1_only = bool(os.environ.get("COEVOL_RANK1_ONLY"))
    psl_all = {}

    def emit_m_chunk(rp, m_lo, m_hi):
        if rp not in psl_all:
            psl_all[rp] = [
                bank.tile([128, CB], F32, tag="bank", name=f"psl{rp}_{g}")[:, 0:2 * L]
                for g in range(4)]
        psl = psl_all[rp]
        prp4 = pair_sb[rp][:].rearrange("p (r l m) -> p r l m", r=2, m=J)
        for m in ([] if rank1_only else range(m_lo, m_hi)):
            for g in range(4):
                nc.tensor.matmul(
                    psl[g][:],
                    wrep_t[32 * g:32 * (g + 1), m * F:(m + 1) * F],
                    prp4[32 * g:32 * (g + 1), :, :, m],
                    start=(m == 0), stop=(no_rank1 and m == J - 1),
                    tile_position=(32 * g, 0),
                    skip_group_check=True)

    def emit_m_loop(rp):
        emit_m_chunk(rp, 0, J)

    # ---- phase A: pair matmuls + bf16 copies + sumsq reduction ----
    for rt in range(NRT):
        rp, rt2 = rt // 2, rt % 2
        for cb in range(NCB):
            pp = bank.tile([128, CB], F32, tag="bank")
            for k in range(NK):
                nc.tensor.matmul(
                    pp[:],
                    a_t[k][:, rt * 128:(rt + 1) * 128],
                    b_t[k][cb][:],
                    start=(k == 0),
                    stop=(k == NK - 1),
                )
            # psum -> sbuf bf16 copy (Linear input)
            nc.scalar.activation(
                pair_sb[rp][:, rt2 * L * J + cb * CB: rt2 * L * J + (cb + 1) * CB],
                pp[:], ACTF.Copy)
        # squares (bf16) on DVE; in-place fold-tree m-reduce (bf16) on DVE
        sq_bf = work.tile([128, L * J], BF16, tag="sq_bf", bufs=2)
        pslice = pair_sb[rp][:, rt2 * L * J:(rt2 + 1) * L * J]
        if rt % 3 == 2:
            nc.scalar.activation(sq_bf[:], pslice, ACTF.Square)
        else:
            nc.vector.scalar_tensor_tensor(
                sq_bf[:], pslice, 1.0, pslice, op0=ALU.mult, op1=ALU.mult)
        sqv = sq_bf[:].rearrange("p (l m) -> p l m", m=J)
        half = J // 2
        while half > 1:
            nc.vector.tensor_add(
                sqv[:, :, 0:half], sqv[:, :, 0:half], sqv[:, :, half:2 * half])
            half //= 2
        ssq_part = work.tile([128, L], F32, tag="ssq_part", bufs=2)
        nc.vector.tensor_add(ssq_part[:], sqv[:, :, 0], sqv[:, :, 1])
        nc.tensor.matmul(ssq_ps, bones_t[:, rt * LI:(rt + 1) * LI], ssq_part[:],
                         start=(rt == 0), stop=(rt == NRT - 1),
                         skip_group_check=True)
        if rt >= 2 and not os.environ.get("COEVOL_NO_ILV"):
            # drip rp0's Linear m-loop between the remaining pair row-tiles
            emit_m_chunk(0, 8 * (rt - 2), 8 * (rt - 1))

    # ---- finalize stats wholesale on [24, L] ----
    mean24 = work.tile([LI, L], F32, tag="mean24")
    nc.vector.tensor_scalar_mul(mean24[:], mean_ps, 1.0 / D2)
    mean2 = work.tile([LI, L], F32, tag="mean2")
    nc.vector.tensor_mul(mean2[:], mean24[:], mean24[:])
    var24 = work.tile([LI, L], F32, tag="var24")
    nc.vector.scalar_tensor_tensor(
        var24[:], ssq_ps, 1.0 / D2, mean2[:], op0=ALU.mult, op1=ALU.subtract)
    std24 = work.tile([LI, L], F32, tag="std24")
    nc.scalar.activation(std24[:], var24[:], ACTF.Sqrt, bias=eps24[:])
    invstd24 = work.tile([LI, L], F32, tag="invstd24")
    nc.vector.reciprocal(invstd24[:], std24[:])
    mneg24 = work.tile([LI, L], BF16, tag="mneg24")
    nc.vector.tensor_scalar_mul(mneg24[:], mean24[:], -1.0)
    stdbf24 = work.tile([LI, L], BF16, tag="stdbf24")
    nc.vector.tensor_copy(stdbf24[:], std24[:])
    for g in range(4):
        nc.sync.dma_start(
            stage2[32 * g:32 * g + 1, 0:LI * L].rearrange("o (i l) -> o i l", i=LI),
            mneg24[:])
        nc.sync.dma_start(
            stage2[32 * g + 1:32 * g + 2, 0:LI * L].rearrange("o (i l) -> o i l", i=LI),
            stdbf24[:])
    nc.sync.dma_start(stage_inv[0:1, :].rearrange("o (i l) -> o i l", i=LI), invstd24[:])

    if os.environ.get("COEVOL_PHASE") == "A":
        # keep phase A live through its outputs; fill the rest with zeros
        nc.sync.dma_start(y[0:1, :], stage_inv[:])
        nc.sync.dma_start(y[1:3, 0:LI * L // 2], stage2[0:2, :].bitcast(F32)[:, 0:LI * L // 2])
        nc.sync.dma_start(y[3:128, :],
                          pair_sb[0][:].bitcast(F32)[3:128, 0:LI * L])
        return

    # ---- phase C: remaining m-loops + rank-1 + invstd scale ----
    st4 = stage2[:].rearrange("p (h g l) -> p h g l", g=4, l=L)
    for rp in range(NRP):
        if rp not in psl_all:
            emit_m_loop(rp)
        psl = psl_all[rp]
        # rank-1 corrections: s x (-mean) + bconst x std
        if not no_rank1:
            for g in range(4):
                nc.tensor.matmul(
                    psl[g][:],
                    sb2_t[32 * g:32 * (g + 1), :],
                    st4[32 * g:32 * (g + 1), 2 * rp:2 * rp + 2, g, :],
                    start=rank1_only, stop=True, tile_position=(32 * g, 0),
                    skip_group_check=True)
        # epilogue: scale columns by invstd[t], write out in [f, t] layout
        for g in range(4):
            out_sb = work.tile([128, 2 * L], F32, tag="out_sb", bufs=3)
            if os.environ.get("COEVOL_NO_PBCAST"):
                nc.vector.tensor_copy(out_sb[:], psl[g][:])
            else:
                inv_bc = work.tile([128, 2 * L], F32, tag="inv_bc", bufs=3)
                for rt2 in range(2):
                    i = (2 * rp + rt2) * 4 + g
                    nc.gpsimd.partition_broadcast(
                        inv_bc[:, rt2 * L:(rt2 + 1) * L],
                        stage_inv[0:1, i * L:(i + 1) * L])
                nc.vector.tensor_mul(out_sb[:], psl[g][:], inv_bc[:])
            for rt2 in range(2):
                i = (2 * rp + rt2) * 4 + g
                nc.sync.dma_start(y[:, i * L:(i + 1) * L], out_sb[:, rt2 * L:(rt2 + 1) * L])


def build_program():
    nc = bacc.Bacc("TRN2", target_bir_lowering=False, debug=False,
                   num_devices=NCORES)
    xa = nc.dram_tensor("xa", [N, LI * J], F32R, kind="ExternalInput").ap()
    xb = nc.dram_tensor("xb", [N, L * J], F32R, kind="ExternalInput").ap()
    wrep = nc.dram_tensor("wrep", [128, J * F], BF16, kind="ExternalInput").ap()
    sb2 = nc.dram_tensor("sb2", [128, F], BF16, kind="ExternalInput").ap()
    bones = nc.dram_tensor("bones", [128, NRT * LI], F32, kind="ExternalInput").ap()
    y = nc.dram_tensor("y", [F, LI * L], F32, kind="ExternalOutput").ap()

    reps = int(os.environ.get("COEVOL_REPS", "1"))
    with tile.TileContext(nc) as tc:
        for _ in range(reps):
            with ExitStack() as ctx:
                build_kernel(ctx, tc, xa, xb, wrep, sb2, bones, y)
    nc.compile()
    return nc


def host_inputs(x_down, x_down_w, a_2, b_2, W, b):
    """Host-side prep: reshapes + weight prepacking. Returns per-core input maps."""
    A2 = np.ascontiguousarray(x_down.reshape(N, L * J).astype(np.float32))
    B2 = np.ascontiguousarray(x_down_w.reshape(N, L * J).astype(np.float32))
    Wp = (a_2.astype(np.float64)[:, None] * W.astype(np.float64))
    s_row = Wp.sum(axis=0)
    bconst = b_2.astype(np.float64) @ W.astype(np.float64) + b.astype(np.float64)
    wrep = np.tile(Wp.reshape(J, J * F), (4, 1)).astype(ml_dtypes.bfloat16)
    sb2 = np.zeros((128, F), dtype=ml_dtypes.bfloat16)
    for g in range(4):
        sb2[32 * g] = s_row.astype(ml_dtypes.bfloat16)
        sb2[32 * g + 1] = bconst.astype(ml_dtypes.bfloat16)
    # per-row-tile j-reduction indicators: bones[:, rt*LI + i'] = 1 where the
    # partition belongs to group g and i' == 4*rt + g
    bones = np.zeros((128, NRT * LI), dtype=np.float32)
    for rt in range(NRT):
        for g in range(4):
            bones[32 * g:32 * (g + 1), rt * LI + 4 * rt + g] = 1.0
    in_maps = []
    for c in range(NCORES):
        in_maps.append({
            "xa": np.ascontiguousarray(A2[:, c * LI * J:(c + 1) * LI * J]),
            "xb": B2,
            "wrep": wrep,
            "sb2": sb2,
            "bones": bones,
        })
    return in_maps


_NC_CACHE = {}


def _get_program():
    if "nc" not in _NC_CACHE:
        _NC_CACHE["nc"] = build_program()
    return _NC_CACHE["nc"]


def kernel(**inputs) -> np.ndarray:
    nc = _get_program()
    inputs = {k: np.asarray(v) for k, v in inputs.items()}
    in_maps = host_inputs(**inputs)
    trace = bool(int(os.environ.get("COEVOL_TRACE", "0")))
    res = run_bass_kernel_spmd(nc, in_maps, list(range(NCORES)), trace=trace)
    if trace:
        _NC_CACHE["last_result"] = res
    # per-core y is [F, LI*L]; unshard to (B, L, L, F)
    slabs = [res.results[c]["y"].reshape(F, LI, L).transpose(1, 2, 0)
             for c in range(NCORES)]
    return np.concatenate(slabs, axis=0).reshape(B, L, L, F)
